# revision 39
# baseline (speedup 1.0000x reference)
"""DiT attention (B=2, S=2048, H=2048, 16 heads / 4 KV heads, RoPE) on 8 trn2
NeuronCores.

Sharding: core c -> batch b = c//4, head-group g = c%4 (q-heads 4g..4g+3 and
kv-head g).  Each core computes its heads' attention for its batch plus the
partial output projection over its 512 O-columns; the host sums the 4 partials
per batch and adds the output bias.

All matmuls run in float32r (full-rate fp32 with hardware-rounded operands).
Phase-2 (attention) pools are allocated disjointly while phase-1 pools are
live, so attention matmuls can backfill phase-1's DMA-bound bubbles.
"""
import sys

if '/opt/trn_rl_repo' not in sys.path:
    sys.path.insert(0, '/opt/trn_rl_repo')

from contextlib import ExitStack

import numpy as np

import concourse.bass as bass  # noqa: F401  (registers types)
import concourse.tile as tile
import concourse.mybir as mybir
from concourse import bacc, bass_utils

B, S, H = 2, 2048, 2048
NH, NKV, HD = 16, 4, 128
P = 128
SCALING = HD ** -0.5
KO = H // P          # 16 contraction tiles for the projections
CH1 = 256            # phase-1 sequence chunk
NCH1 = S // CH1      # 8
CH2 = 512            # phase-2/3 q / y chunk
NCH2 = S // CH2      # 4
NQH = NH // NKV      # 4 q heads per core
KT = S // P          # 16 key tiles
F32 = mybir.dt.float32
F32R = mybir.dt.float32r
AF = mybir.ActivationFunctionType

_NC_CACHE = []


def _build_nc():
    nc = bacc.Bacc("TRN2", target_bir_lowering=False, debug=False,
                   enable_asserts=True, num_devices=8)
    xt = nc.dram_tensor("xt", [H, S], F32R, kind="ExternalInput").ap()
    wq = nc.dram_tensor("wq", [H, NQH * HD], F32R, kind="ExternalInput").ap()
    wk = nc.dram_tensor("wk", [H, HD], F32R, kind="ExternalInput").ap()
    wv = nc.dram_tensor("wv", [H, HD], F32R, kind="ExternalInput").ap()
    wo = nc.dram_tensor("wo", [NQH * HD, H], F32R, kind="ExternalInput").ap()
    cosT = nc.dram_tensor("cosT", [HD, S], F32, kind="ExternalInput").ap()
    sinT = nc.dram_tensor("sinT", [HD, S], F32, kind="ExternalInput").ap()
    bqT = nc.dram_tensor("bqT", [HD, NQH], F32, kind="ExternalInput").ap()
    bkT = nc.dram_tensor("bkT", [HD, 1], F32, kind="ExternalInput").ap()
    bvT = nc.dram_tensor("bvT", [HD, 1], F32, kind="ExternalInput").ap()
    # [:, 0:128] identity (PE transpose), [:, 128] all-ones (denominator)
    ones = nc.dram_tensor("ones", [P, P + 1], F32R, kind="ExternalInput").ap()
    y = nc.dram_tensor("y", [S, H], F32, kind="ExternalOutput").ap()

    with tile.TileContext(nc) as tc, ExitStack() as ctx:
        const = ctx.enter_context(tc.tile_pool(name="const", bufs=1))
        wq_sb = const.tile([P, KO, NQH * HD], F32R)
        wk_sb = const.tile([P, KO, HD], F32R)
        wv_sb = const.tile([P, KO, HD], F32R)
        bq_sb = const.tile([HD, NQH], F32)
        bk_sb = const.tile([HD, 1], F32)
        bv_sb = const.tile([HD, 1], F32)
        on_sb = const.tile([P, P + 1], F32R)
        # small tensors first; wq/xt chunk 0 are interleaved in compute order
        nc.sync.dma_start(bq_sb[:], bqT)
        nc.sync.dma_start(bk_sb[:], bkT)
        nc.sync.dma_start(bv_sb[:], bvT)
        nc.sync.dma_start(on_sb[:], ones)
        wq_t = wq.rearrange("(ko p) m -> p ko m", p=P)

        res = ctx.enter_context(tc.tile_pool(name="res", bufs=1))
        qrop = res.tile([HD, NQH, S], F32R)   # Q^T roped; reused as O^T later
        krop = res.tile([HD, S], F32R)        # K^T roped
        v_sb = res.tile([P, KT, HD], F32R)    # V natural, k-tiled

        p1_stack = ExitStack()
        csp = p1_stack.enter_context(tc.tile_pool(name="csp", bufs=1))
        cos_sb = csp.tile([HD, S], F32)
        sin_sb = csp.tile([HD, S], F32)
        xtp = p1_stack.enter_context(tc.tile_pool(name="xtp", bufs=3))
        p1s = p1_stack.enter_context(tc.tile_pool(name="p1s", bufs=3))
        p1ps = p1_stack.enter_context(
            tc.tile_pool(name="p1ps", bufs=2, space="PSUM"))
        tpp = p1_stack.enter_context(
            tc.tile_pool(name="tpp", bufs=2, space="PSUM"))

        # ---------------- phase 1: projections + RoPE ----------------
        for c in range(NCH1):
            scol = slice(c * CH1, (c + 1) * CH1)
            xt_c = xtp.tile([P, KO, CH1], F32R, tag="xt")
            xt_r = xt[:, scol].rearrange("(ko p) s -> p ko s", p=P)
            if c == 0:   # interleave with wq in compute order
                ko0 = 0
                for gsz in (1, 1, 2, 4, 8):
                    ksl = slice(ko0, ko0 + gsz)
                    nc.sync.dma_start(xt_c[:, ksl, :], xt_r[:, ksl, :])
                    nc.sync.dma_start(wq_sb[:, ksl, :], wq_t[:, ksl, :])
                    ko0 += gsz
                nc.sync.dma_start(
                    wk_sb[:], wk.rearrange("(ko p) m -> p ko m", p=P))
                nc.sync.dma_start(
                    wv_sb[:], wv.rearrange("(ko p) m -> p ko m", p=P))
                nc.gpsimd.dma_start(cos_sb[:], cosT)
                nc.gpsimd.dma_start(sin_sb[:], sinT)
            else:
                # separate SWDGE queue: don't contend with weight loads
                nc.gpsimd.dma_start(xt_c[:], xt_r)
            # Q (4 heads), K, V^T - all [d, s] with the same moving xt
            for h in range(NQH + 2):
                ps = p1ps.tile([P, CH1], F32, tag="qk")
                for ko in range(KO):
                    if h < NQH:
                        lhsT = wq_sb[:, ko, h * HD:(h + 1) * HD]
                    elif h == NQH:
                        lhsT = wk_sb[:, ko, :]
                    else:
                        lhsT = wv_sb[:, ko, :]
                    nc.tensor.matmul(ps[:], lhsT, xt_c[:, ko, :],
                                     start=(ko == 0), stop=(ko == KO - 1))
                if h <= NQH:
                    bias = bq_sb[:, h:h + 1] if h < NQH else bk_sb[:, 0:1]
                    qf = p1s.tile([HD, CH1], F32, tag="qf")
                    nc.scalar.activation(qf[:], ps[:], AF.Identity,
                                         bias=bias, scale=1.0)
                    qs = p1s.tile([HD, CH1], F32, tag="qs")
                    nc.sync.dma_start(qs[0:64, :], qf[64:128, :])
                    nc.sync.dma_start(qs[64:128, :], qf[0:64, :])
                    t2 = p1s.tile([HD, CH1], F32, tag="t2")
                    nc.vector.tensor_mul(t2[:], qs[:], sin_sb[:, scol])
                    nc.vector.tensor_mul(qf[:], qf[:], cos_sb[:, scol])
                    dst = qrop[:, h, scol] if h < NQH else krop[:, scol]
                    nc.vector.tensor_add(dst, qf[:], t2[:])
                else:
                    # V^T -> (bias-add) -> PE-transpose to V natural
                    vt = p1s.tile([HD, CH1], F32R, tag="vt")
                    nc.scalar.activation(vt[:], ps[:], AF.Identity,
                                         bias=bv_sb[:, 0:1], scale=1.0)
                    for st in range(CH1 // P):
                        kt = c * (CH1 // P) + st
                        tps = tpp.tile([P, P], F32R, tag="tp")
                        nc.tensor.transpose(
                            tps[:], vt[:, st * P:(st + 1) * P],
                            on_sb[:, 0:P])
                        nc.vector.tensor_copy(v_sb[:, kt, :], tps[:])

        # release phase-1 pools; later pools reuse their space
        p1_stack.close()

        # ---------------- phase 2 + 3 interleaved ----------------
        wop = ctx.enter_context(tc.tile_pool(name="wop", bufs=1))
        wo_sb = wop.tile([P, NQH, H], F32R)
        nc.sync.dma_start(wo_sb[:], wo.rearrange("(h p) n -> p h n", p=P))
        p2s = ctx.enter_context(tc.tile_pool(name="p2s", bufs=4))
        p2sm = ctx.enter_context(tc.tile_pool(name="p2sm", bufs=2))
        stp = ctx.enter_context(tc.tile_pool(name="stp", bufs=3, space="PSUM"))
        opp = ctx.enter_context(tc.tile_pool(name="opp", bufs=2, space="PSUM"))
        smp = ctx.enter_context(tc.tile_pool(name="smp", bufs=1, space="PSUM"))
        p3s = ctx.enter_context(tc.tile_pool(name="p3s", bufs=3))
        ypp = ctx.enter_context(tc.tile_pool(name="yp", bufs=2, space="PSUM"))
        # qc outer so phase-3 q-tiles become ready early (PE backfill)
        for qc in range(NCH2):
            qsl = slice(qc * CH2, (qc + 1) * CH2)
            for h in range(NQH):
                o_ps = opp.tile([HD, CH2], F32, tag="o")
                s_ps = smp.tile([1, CH2], F32, tag="sm")
                # software-pipelined: scores matmul runs one kt ahead so the
                # PE has work while ACT computes exp(kt)
                sts = [None] * KT
                st0 = stp.tile([P, CH2], F32, tag="st")
                sts[0] = st0
                nc.tensor.matmul(sts[0][:], krop[:, 0:P],
                                 qrop[:, h, qsl], start=True, stop=True)
                for kt in range(KT):
                    if kt + 1 < KT:
                        stn = stp.tile([P, CH2], F32, tag="st")
                        sts[kt + 1] = stn
                        nc.tensor.matmul(sts[kt + 1][:],
                                         krop[:, (kt + 1) * P:(kt + 2) * P],
                                         qrop[:, h, qsl],
                                         start=True, stop=True)
                    pt = p2s.tile([P, CH2], F32R, tag="pt")
                    nc.scalar.activation(pt[:], sts[kt][:], AF.Exp,
                                         scale=SCALING)
                    sts[kt] = None
                    nc.tensor.matmul(o_ps[:], v_sb[:, kt, :], pt[:],
                                     start=(kt == 0), stop=(kt == KT - 1))
                    nc.tensor.matmul(s_ps[:], on_sb[:, P:P + 1], pt[:],
                                     start=(kt == 0), stop=(kt == KT - 1))
                rec = p2sm.tile([1, CH2], F32, tag="rec")
                nc.vector.reciprocal_approx_fast(rec[:], s_ps[:])
                rb = p2sm.tile([P, CH2], F32, tag="rb")
                nc.gpsimd.partition_broadcast(rb[:], rec[:])
                # normalized O^T overwrites the spent Q^T slice
                nc.vector.tensor_mul(qrop[:, h, qsl], o_ps[:], rb[:])

            # ---- phase 3 for the q-tiles this qc completed ----
            for qt in range(qc * (CH2 // P), (qc + 1) * (CH2 // P)):
                for ycn in range(NCH2):
                    ysl = slice(ycn * CH2, (ycn + 1) * CH2)
                    y_ps = ypp.tile([P, CH2], F32, tag="y")
                    for hh in range(NQH):
                        nc.tensor.matmul(y_ps[:],
                                         qrop[:, hh, qt * P:(qt + 1) * P],
                                         wo_sb[:, hh, ysl],
                                         start=(hh == 0),
                                         stop=(hh == NQH - 1))
                    y_sb = p3s.tile([P, CH2], F32, tag="ysb")
                    nc.vector.tensor_copy(y_sb[:], y_ps[:])
                    nc.sync.dma_start(y[qt * P:(qt + 1) * P, ysl], y_sb[:])

    nc.compile()
    return nc


def _get_nc():
    if not _NC_CACHE:
        _NC_CACHE.append(_build_nc())
    return _NC_CACHE[0]


def kernel(**inputs) -> np.ndarray:
    hs = np.ascontiguousarray(np.asarray(inputs["hidden_states"], np.float32))
    cos = np.asarray(inputs["cos"], np.float32)
    sin = np.asarray(inputs["sin"], np.float32)
    Wq = np.asarray(inputs["Wq"], np.float32)
    bq = np.asarray(inputs["bq"], np.float32)
    Wk = np.asarray(inputs["Wk"], np.float32)
    bk = np.asarray(inputs["bk"], np.float32)
    Wv = np.asarray(inputs["Wv"], np.float32)
    bv = np.asarray(inputs["bv"], np.float32)
    Wo = np.asarray(inputs["Wo"], np.float32)
    bo = np.asarray(inputs["bo"], np.float32)

    nc = _get_nc()

    XT = [np.ascontiguousarray(hs[b].T) for b in range(B)]
    cosT = [np.ascontiguousarray(cos[b].T) for b in range(B)]
    sinTs = []
    for b in range(B):
        st = np.ascontiguousarray(sin[b].T)
        st[0:64] = -st[0:64]          # fold rotate_half sign into the table
        sinTs.append(st)
    ones = np.zeros((P, P + 1), np.float32)
    ones[:, 0:P] = np.eye(P, dtype=np.float32)
    ones[:, P] = 1.0

    in_maps = []
    for c in range(8):
        b, g = c // 4, c % 4
        in_maps.append({
            "xt": XT[b],
            "wq": np.ascontiguousarray(Wq[:, g * NQH * HD:(g + 1) * NQH * HD]),
            "wk": np.ascontiguousarray(Wk[:, g * HD:(g + 1) * HD]),
            "wv": np.ascontiguousarray(Wv[:, g * HD:(g + 1) * HD]),
            "wo": np.ascontiguousarray(Wo[g * NQH * HD:(g + 1) * NQH * HD, :]),
            "cosT": cosT[b],
            "sinT": sinTs[b],
            "bqT": np.ascontiguousarray(
                bq[g * NQH * HD:(g + 1) * NQH * HD].reshape(NQH, HD).T),
            "bkT": np.ascontiguousarray(
                bk[g * HD:(g + 1) * HD].reshape(1, HD).T),
            "bvT": np.ascontiguousarray(
                bv[g * HD:(g + 1) * HD].reshape(1, HD).T),
            "ones": ones,
        })

    res = bass_utils.run_bass_kernel_spmd(nc, in_maps, core_ids=list(range(8)))

    out = np.empty((B, S, H), np.float32)
    for b in range(B):
        acc = res.results[4 * b]["y"].copy()
        for g in range(1, 4):
            acc += res.results[4 * b + g]["y"]
        out[b] = acc + bo[None, :]
    return out
